# revision 3
# baseline (speedup 1.0000x reference)
"""Trainium2 Bass kernel for a single-step attention-decoder RNN
(embedding lookup -> additive attention -> combine+relu -> GRU cell ->
vocab projection -> log_softmax), tensor-parallel over the vocab dim
across 8 NeuronCores.

Sharding strategy (all host-side prep, one shared SPMD NEFF):
  - embedding lookup is a host-side row slice; the 4KB row replicates.
  - comb_W row-sharded (each core computes 128 of 1024 outputs).
  - GRU Wih/Whh column-sharded; one 24KB AllReduce combines partials,
    then every core computes the full h_new (1024 elems) redundantly.
  - out_W row-sharded (6400 padded vocab rows per core); local
    max/sum-exp stats are AllGathered (8x2 floats) to form the global
    logsumexp; each core writes its log-prob shard.
All per-core weight shards are pre-transposed on the host so every
matvec runs on the tensor engine with K on partitions and perfectly
contiguous DMA loads.
"""
import sys

sys.path.insert(0, "/opt/trn_rl_repo")

import numpy as np

import concourse.bacc as bacc
import concourse.mybir as mybir
import concourse.tile as tile
from concourse.bass_utils import run_bass_kernel_spmd

H = 1024
L = 18
V = 50257
N_CORES = 8
VP = 6400          # padded vocab rows per core (50 m-tiles of 128)
MT = VP // 128     # 50 m-tiles
KC = H // 128      # 8 contraction chunks over H
G3 = 3 * H // 128  # 24 gate-row tiles (3 gates x 8 chunks)
F32 = mybir.dt.float32
NEG_PAD = -1e30

_STATE: dict = {}


def _build_nc():
    nc = bacc.Bacc("TRN2", target_bir_lowering=False, debug=False,
                   num_devices=N_CORES)

    def din(name, shape):
        return nc.dram_tensor(name, shape, F32, kind="ExternalInput")

    def dout(name, shape):
        return nc.dram_tensor(name, shape, F32, kind="ExternalOutput")

    e_col = din("e_col", [128, KC])
    h_col = din("h_col", [128, KC])
    h_slice = din("h_slice", [128, 1])
    enc = din("enc", [L, H])
    attn_wt = din("attn_wt", [128, 16 * L])
    attn_b_col = din("attn_b_col", [L, 1])
    comb_wt = din("comb_wt", [128, 16 * 128])
    comb_b_col = din("comb_b_col", [128, 1])
    wih_t = din("wih_t", [128, G3 * 128])
    whh_t = din("whh_t", [128, G3 * 128])
    bih_col = din("bih_col", [128, G3])
    bhh_col = din("bhh_col", [128, G3])
    out_wt = din("out_wt", [H, VP])
    out_b_col = din("out_b_col", [128, MT])
    ident = din("ident", [128, 128])
    ones_row = din("ones_row", [1, 128])

    logp_out = dout("logp", [MT, 128])
    hnew_out = dout("hnew", [128, KC])
    attnw_out = dout("attnw", [1, L])

    ar_in = nc.dram_tensor("ar_in", [128, 2 * G3], F32)
    ar_out = nc.dram_tensor("ar_out", [128, 2 * G3], F32, addr_space="Shared")
    ag_in = nc.dram_tensor("ag_in", [1, 2], F32)
    ag_out = nc.dram_tensor("ag_out", [N_CORES, 2], F32, addr_space="Shared")

    RG = [list(range(N_CORES))]

    with tile.TileContext(nc) as tc:
        with (
            tc.tile_pool(name="const", bufs=1) as const,
            tc.tile_pool(name="wpool", bufs=4) as wpool,
            tc.tile_pool(name="sb1", bufs=1) as sb1,
            tc.tile_pool(name="pps", bufs=3, space="PSUM") as pps,
            tc.tile_pool(name="plog", bufs=1, space="PSUM") as plog,
        ):
            def cload(dram, shape, tag):
                t = const.tile(shape, F32, tag=tag, name=tag)
                nc.sync.dma_start(t[:], dram.ap())
                return t

            # ---- small constants (DMA'd first, in dependency order) ----
            e_sb = cload(e_col, [128, KC], "e_sb")
            h_sb = cload(h_col, [128, KC], "h_sb")
            hs_sb = cload(h_slice, [128, 1], "hs_sb")
            aw_sb = cload(attn_wt, [128, 16 * L], "aw_sb")
            ab_sb = cload(attn_b_col, [L, 1], "ab_sb")
            enc_sb = cload(enc, [L, H], "enc_sb")
            id_sb = cload(ident, [128, 128], "id_sb")
            on_sb = cload(ones_row, [1, 128], "on_sb")
            cw_sb = cload(comb_wt, [128, 16 * 128], "cw_sb")
            cb_sb = cload(comb_b_col, [128, 1], "cb_sb")
            wih_sb = cload(wih_t, [128, G3 * 128], "wih_sb")
            whh_sb = cload(whh_t, [128, G3 * 128], "whh_sb")
            bih_sb = cload(bih_col, [128, G3], "bih_sb")
            bhh_sb = cload(bhh_col, [128, G3], "bhh_sb")
            ob_sb = cload(out_b_col, [128, MT], "ob_sb")

            # ---- attention scores: [L, 1] = attn_W @ concat(e, h) + b ----
            scores_ps = pps.tile([L, 1], F32, tag="tmp", name="scores_ps")
            for n in range(16):
                rhs = e_sb[:, n:n + 1] if n < 8 else h_sb[:, n - 8:n - 7]
                nc.tensor.matmul(scores_ps[:], aw_sb[:, L * n:L * (n + 1)],
                                 rhs, start=(n == 0), stop=(n == 15))
            scores_sb = sb1.tile([L, 1], F32, tag="scores_sb", name="scores_sb")
            nc.vector.tensor_add(scores_sb[:], scores_ps[:], ab_sb[:])

            # softmax over the 18 scores (transpose to one partition)
            st_ps = pps.tile([1, L], F32, tag="tmp", name="st_ps")
            nc.tensor.matmul(st_ps[:], scores_sb[:], id_sb[:L, :L],
                             is_transpose=True)
            st_sb = sb1.tile([1, L], F32, tag="st_sb", name="st_sb")
            nc.vector.tensor_copy(st_sb[:], st_ps[:])
            negm = sb1.tile([1, 1], F32, tag="negm", name="negm")
            nc.vector.tensor_reduce(negm[:], st_sb[:],
                                    axis=mybir.AxisListType.X,
                                    op=mybir.AluOpType.max, negate=True)
            expv = sb1.tile([1, L], F32, tag="expv", name="expv")
            nc.scalar.activation(expv[:], st_sb[:],
                                 mybir.ActivationFunctionType.Exp,
                                 bias=negm[:], scale=1.0)
            ssum = sb1.tile([1, 1], F32, tag="ssum", name="ssum")
            nc.vector.reduce_sum(ssum[:], expv[:],
                                 axis=mybir.AxisListType.X)
            sinv = sb1.tile([1, 1], F32, tag="sinv", name="sinv")
            nc.vector.reciprocal(sinv[:], ssum[:])
            attnw = sb1.tile([1, L], F32, tag="attnw", name="attnw")
            nc.vector.tensor_scalar_mul(attnw[:], expv[:], sinv[:])
            nc.gpsimd.dma_start(attnw_out.ap(), attnw[:])

            # attn_applied[128j+m] = sum_l attnw[l] * enc[l, 128j+m]
            awc_ps = pps.tile([L, 1], F32, tag="tmp", name="awc_ps")
            nc.tensor.matmul(awc_ps[:], attnw[:], id_sb[:1, :1],
                             is_transpose=True)
            awc_sb = sb1.tile([L, 1], F32, tag="awc_sb", name="awc_sb")
            nc.vector.tensor_copy(awc_sb[:], awc_ps[:])
            aa_ps = pps.tile([128, KC], F32, tag="tmp", name="aa_ps")
            for j in range(KC):
                nc.tensor.matmul(aa_ps[:, j:j + 1],
                                 enc_sb[:, 128 * j:128 * (j + 1)], awc_sb[:],
                                 start=True, stop=True)
            aa_sb = sb1.tile([128, KC], F32, tag="aa_sb", name="aa_sb")
            nc.vector.tensor_copy(aa_sb[:], aa_ps[:])

            # ---- combine + relu: x_c = relu(comb_W_c @ concat(e, aa) + b) ----
            x_ps = pps.tile([128, 1], F32, tag="tmp", name="x_ps")
            for n in range(16):
                rhs = e_sb[:, n:n + 1] if n < 8 else aa_sb[:, n - 8:n - 7]
                nc.tensor.matmul(x_ps[:], cw_sb[:, 128 * n:128 * (n + 1)],
                                 rhs, start=(n == 0), stop=(n == 15))
            x_sb = sb1.tile([128, 1], F32, tag="x_sb", name="x_sb")
            nc.scalar.activation(x_sb[:], x_ps[:],
                                 mybir.ActivationFunctionType.Relu,
                                 bias=cb_sb[:], scale=1.0)

            # ---- GRU partial gates: gi_c = Wih[:, cols_c] @ x_c, etc. ----
            gi_ps = pps.tile([128, G3], F32, tag="gi", name="gi_ps", bufs=1)
            gh_ps = pps.tile([128, G3], F32, tag="gh", name="gh_ps", bufs=1)
            for t in range(G3):
                nc.tensor.matmul(gi_ps[:, t:t + 1],
                                 wih_sb[:, 128 * t:128 * (t + 1)], x_sb[:],
                                 start=True, stop=True)
            for t in range(G3):
                nc.tensor.matmul(gh_ps[:, t:t + 1],
                                 whh_sb[:, 128 * t:128 * (t + 1)], hs_sb[:],
                                 start=True, stop=True)
            part_sb = sb1.tile([128, 2 * G3], F32, tag="part_sb", name="part_sb")
            nc.vector.tensor_copy(part_sb[:, 0:G3], gi_ps[:])
            nc.vector.tensor_copy(part_sb[:, G3:2 * G3], gh_ps[:])

            # AllReduce the partial gate pre-activations (24 KB)
            nc.gpsimd.dma_start(ar_in.ap(), part_sb[:])
            nc.gpsimd.collective_compute(
                "AllReduce", mybir.AluOpType.add, replica_groups=RG,
                ins=[ar_in.ap().opt()], outs=[ar_out.ap().opt()])
            full_sb = sb1.tile([128, 2 * G3], F32, tag="full_sb", name="full_sb")
            nc.gpsimd.dma_start(full_sb[:], ar_out.ap())

            # ---- gates + h_new (all cores compute the full 1024 vector) ----
            gib = sb1.tile([128, G3], F32, tag="gib", name="gib")
            nc.vector.tensor_add(gib[:], full_sb[:, 0:G3], bih_sb[:])
            ghb = sb1.tile([128, G3], F32, tag="ghb", name="ghb")
            nc.vector.tensor_add(ghb[:], full_sb[:, G3:2 * G3], bhh_sb[:])
            rz_in = sb1.tile([128, 16], F32, tag="rz_in", name="rz_in")
            nc.vector.tensor_add(rz_in[:], gib[:, 0:16], ghb[:, 0:16])
            rz = sb1.tile([128, 16], F32, tag="rz", name="rz")
            nc.scalar.activation(rz[:], rz_in[:],
                                 mybir.ActivationFunctionType.Sigmoid)
            rh = sb1.tile([128, KC], F32, tag="rh", name="rh")
            nc.vector.tensor_mul(rh[:], rz[:, 0:8], ghb[:, 16:24])
            n_in = sb1.tile([128, KC], F32, tag="n_in", name="n_in")
            nc.vector.tensor_add(n_in[:], gib[:, 16:24], rh[:])
            n_t = sb1.tile([128, KC], F32, tag="n_t", name="n_t")
            nc.scalar.activation(n_t[:], n_in[:],
                                 mybir.ActivationFunctionType.Tanh)
            hmn = sb1.tile([128, KC], F32, tag="hmn", name="hmn")
            nc.vector.tensor_sub(hmn[:], h_sb[:], n_t[:])
            zh = sb1.tile([128, KC], F32, tag="zh", name="zh")
            nc.vector.tensor_mul(zh[:], rz[:, 8:16], hmn[:])
            hnew = sb1.tile([128, KC], F32, tag="hnew", name="hnew")
            nc.vector.tensor_add(hnew[:], n_t[:], zh[:])
            nc.gpsimd.dma_start(hnew_out.ap(), hnew[:])

            # ---- vocab projection: logits[128m+p] over this core's shard ----
            # PSUM start=True zeroes the whole 2KB bank ("zero region"), so
            # per-column starts would wipe sibling columns. Instead memset
            # the bank once and accumulate with start=False throughout.
            logits_ps = plog.tile([128, MT], F32, tag="logits", name="logits_ps")
            nc.vector.memset(logits_ps[:], 0.0)
            for kc in range(KC):
                wt = wpool.tile([128, VP], F32, tag="w", name="wt")
                nc.sync.dma_start(wt[:],
                                  out_wt.ap()[128 * kc:128 * (kc + 1), :])
                for m in range(MT):
                    nc.tensor.matmul(logits_ps[:, m:m + 1],
                                     wt[:, 128 * m:128 * (m + 1)],
                                     hnew[:, kc:kc + 1],
                                     start=False, stop=False,
                                     skip_group_check=True)
            logits_sb = sb1.tile([128, MT], F32, tag="logits_sb",
                                 name="logits_sb")
            nc.vector.tensor_add(logits_sb[:], logits_ps[:], ob_sb[:])

            # ---- local softmax stats ----
            pmax = sb1.tile([128, 1], F32, tag="pmax", name="pmax")
            nc.vector.tensor_reduce(pmax[:], logits_sb[:],
                                    axis=mybir.AxisListType.X,
                                    op=mybir.AluOpType.max)
            pmt_ps = pps.tile([1, 128], F32, tag="tmp", name="pmt_ps")
            nc.tensor.matmul(pmt_ps[:], pmax[:], id_sb[:],
                             is_transpose=True)
            pmt_sb = sb1.tile([1, 128], F32, tag="pmt_sb", name="pmt_sb")
            nc.vector.tensor_copy(pmt_sb[:], pmt_ps[:])
            negmc = sb1.tile([1, 1], F32, tag="negmc", name="negmc")
            nc.vector.tensor_reduce(negmc[:], pmt_sb[:],
                                    axis=mybir.AxisListType.X,
                                    op=mybir.AluOpType.max, negate=True)
            # broadcast -m_c to all partitions
            nmb_ps = pps.tile([128, 1], F32, tag="tmp", name="nmb_ps")
            nc.tensor.matmul(nmb_ps[:], on_sb[:], negmc[:],
                             start=True, stop=True)
            nmb_sb = sb1.tile([128, 1], F32, tag="nmb_sb", name="nmb_sb")
            nc.vector.tensor_copy(nmb_sb[:], nmb_ps[:])
            expl = sb1.tile([128, MT], F32, tag="expl", name="expl")
            nc.scalar.activation(expl[:], logits_sb[:],
                                 mybir.ActivationFunctionType.Exp,
                                 bias=nmb_sb[:], scale=1.0)
            psum_c = sb1.tile([128, 1], F32, tag="psum_c", name="psum_c")
            nc.vector.reduce_sum(psum_c[:], expl[:],
                                 axis=mybir.AxisListType.X)
            pst_ps = pps.tile([1, 128], F32, tag="tmp", name="pst_ps")
            nc.tensor.matmul(pst_ps[:], psum_c[:], id_sb[:],
                             is_transpose=True)
            pst_sb = sb1.tile([1, 128], F32, tag="pst_sb", name="pst_sb")
            nc.vector.tensor_copy(pst_sb[:], pst_ps[:])
            s_c = sb1.tile([1, 1], F32, tag="s_c", name="s_c")
            nc.vector.reduce_sum(s_c[:], pst_sb[:],
                                 axis=mybir.AxisListType.X)

            # stats = [m_c, s_c] -> AllGather -> [8, 2]
            stats = sb1.tile([1, 2], F32, tag="stats", name="stats")
            nc.vector.tensor_scalar_mul(stats[:, 0:1], negmc[:], -1.0)
            nc.vector.tensor_copy(stats[:, 1:2], s_c[:])
            nc.gpsimd.dma_start(ag_in.ap(), stats[:])
            nc.gpsimd.collective_compute(
                "AllGather", mybir.AluOpType.bypass, replica_groups=RG,
                ins=[ag_in.ap().opt()], outs=[ag_out.ap().opt()])
            allst = sb1.tile([N_CORES, 2], F32, tag="allst", name="allst")
            nc.gpsimd.dma_start(allst[:], ag_out.ap())

            # global logsumexp = LSE_c(m_c + ln s_c)
            lns = sb1.tile([N_CORES, 1], F32, tag="lns", name="lns")
            nc.scalar.activation(lns[:], allst[:, 1:2],
                                 mybir.ActivationFunctionType.Ln)
            tvec = sb1.tile([N_CORES, 1], F32, tag="tvec", name="tvec")
            nc.vector.tensor_add(tvec[:], allst[:, 0:1], lns[:])
            tvt_ps = pps.tile([1, N_CORES], F32, tag="tmp", name="tvt_ps")
            nc.tensor.matmul(tvt_ps[:], tvec[:],
                             id_sb[:N_CORES, :N_CORES], is_transpose=True)
            tvt_sb = sb1.tile([1, N_CORES], F32, tag="tvt_sb", name="tvt_sb")
            nc.vector.tensor_copy(tvt_sb[:], tvt_ps[:])
            negM = sb1.tile([1, 1], F32, tag="negM", name="negM")
            nc.vector.tensor_reduce(negM[:], tvt_sb[:],
                                    axis=mybir.AxisListType.X,
                                    op=mybir.AluOpType.max, negate=True)
            exv = sb1.tile([1, N_CORES], F32, tag="exv", name="exv")
            nc.scalar.activation(exv[:], tvt_sb[:],
                                 mybir.ActivationFunctionType.Exp,
                                 bias=negM[:], scale=1.0)
            sex = sb1.tile([1, 1], F32, tag="sex", name="sex")
            nc.vector.reduce_sum(sex[:], exv[:],
                                 axis=mybir.AxisListType.X)
            lg = sb1.tile([1, 1], F32, tag="lg", name="lg")
            nc.scalar.activation(lg[:], sex[:],
                                 mybir.ActivationFunctionType.Ln)
            neglse = sb1.tile([1, 1], F32, tag="neglse", name="neglse")
            # -lse = negM - lg  (lse = lg - negM)
            nc.vector.tensor_sub(neglse[:], negM[:], lg[:])
            nlb_ps = pps.tile([128, 1], F32, tag="tmp", name="nlb_ps")
            nc.tensor.matmul(nlb_ps[:], on_sb[:], neglse[:],
                             start=True, stop=True)
            nlb_sb = sb1.tile([128, 1], F32, tag="nlb_sb", name="nlb_sb")
            nc.vector.tensor_copy(nlb_sb[:], nlb_ps[:])

            # log_probs = logits - lse, transpose, store
            logp_sb = sb1.tile([128, MT], F32, tag="logp_sb", name="logp_sb")
            nc.vector.tensor_scalar_add(logp_sb[:], logits_sb[:],
                                        nlb_sb[:])
            lpt_ps = pps.tile([MT, 128], F32, tag="tmp", name="lpt_ps")
            nc.tensor.matmul(lpt_ps[:], logp_sb[:], id_sb[:],
                             is_transpose=True)
            lpt_sb = sb1.tile([MT, 128], F32, tag="lpt_sb", name="lpt_sb")
            nc.vector.tensor_copy(lpt_sb[:], lpt_ps[:])
            nc.sync.dma_start(logp_out.ap(), lpt_sb[:])

    nc.compile()
    return nc


def _get_nc():
    if "nc" not in _STATE:
        _STATE["nc"] = _build_nc()
    return _STATE["nc"]


def _col(v):
    """[n*128] -> [128, n] with (p, j) = v[128j + p]."""
    return np.ascontiguousarray(v.reshape(-1, 128).T)


def kernel(input_ids, hidden, encoder_outputs, emb, attn_W, attn_b,
           comb_W, comb_b, gru_Wih, gru_Whh, gru_bih, gru_bhh,
           out_W, out_b):
    nc = _get_nc()
    f = lambda x: np.asarray(x, dtype=np.float32)
    input_ids = np.asarray(input_ids)
    idx = int(input_ids.reshape(-1)[0])
    hidden, encoder_outputs = np.asarray(hidden), f(encoder_outputs)
    emb, attn_W, attn_b = np.asarray(emb), f(attn_W), f(attn_b)
    comb_W, comb_b = f(comb_W), f(comb_b)
    gru_Wih, gru_Whh = f(gru_Wih), f(gru_Whh)
    gru_bih, gru_bhh = f(gru_bih), f(gru_bhh)
    out_W, out_b = np.asarray(out_W), np.asarray(out_b)

    e = f(emb[idx])                        # [H]
    h = f(hidden.reshape(-1))              # [H]
    e_col, h_col = _col(e), _col(h)
    attn_wt = np.ascontiguousarray(
        f(attn_W).T.reshape(16, 128, L).transpose(1, 0, 2).reshape(128, 16 * L))
    attn_b_col = f(attn_b).reshape(L, 1)
    ident = np.eye(128, dtype=np.float32)
    ones_row = np.ones((1, 128), dtype=np.float32)
    bih_col, bhh_col = _col(gru_bih), _col(gru_bhh)

    # padded out_W / out_b, transposed per shard
    ow = f(out_W)
    ob = f(out_b)
    ow_pad = np.full((N_CORES * VP, H), 0.0, dtype=np.float32)
    ow_pad[:V] = ow
    ob_pad = np.full(N_CORES * VP, NEG_PAD, dtype=np.float32)
    ob_pad[:V] = ob

    in_maps = []
    for c in range(N_CORES):
        rows = slice(128 * c, 128 * (c + 1))
        cols = slice(128 * c, 128 * (c + 1))
        comb_wt = np.ascontiguousarray(
            comb_W[rows, :].T.reshape(16, 128, 128)
            .transpose(1, 0, 2).reshape(128, 16 * 128))
        in_maps.append({
            "e_col": e_col, "h_col": h_col,
            "h_slice": np.ascontiguousarray(h[cols].reshape(128, 1)),
            "enc": encoder_outputs,
            "attn_wt": attn_wt, "attn_b_col": attn_b_col,
            "comb_wt": comb_wt,
            "comb_b_col": comb_b[rows].reshape(128, 1),
            "wih_t": np.ascontiguousarray(gru_Wih[:, cols].T),
            "whh_t": np.ascontiguousarray(gru_Whh[:, cols].T),
            "bih_col": bih_col, "bhh_col": bhh_col,
            "out_wt": np.ascontiguousarray(ow_pad[VP * c:VP * (c + 1)].T),
            "out_b_col": _col(ob_pad[VP * c:VP * (c + 1)]),
            "ident": ident, "ones_row": ones_row,
        })

    res = run_bass_kernel_spmd(nc, in_maps, core_ids=list(range(N_CORES)),
                               **_STATE.get("run_kwargs", {}))
    _STATE["last_results"] = res

    logp = np.concatenate(
        [res.results[c]["logp"].reshape(-1) for c in range(N_CORES)])[:V]
    h_new = np.ascontiguousarray(res.results[0]["hnew"].T).reshape(-1)
    attnw = res.results[0]["attnw"].reshape(1, L)
    return (logp.reshape(1, V).astype(np.float32),
            h_new.reshape(1, 1, H).astype(np.float32),
            attnw.astype(np.float32))


# revision 6
# speedup vs baseline: 1.2582x; 1.2582x over previous
"""Trainium2 Bass kernel for a single-step attention-decoder RNN
(embedding lookup -> additive attention -> combine+relu -> GRU cell ->
vocab projection -> log_softmax), tensor-parallel over the vocab dim
across 8 NeuronCores.

Sharding strategy (all host-side prep, one shared SPMD NEFF):
  - embedding lookup is a host-side row slice; the 4KB row replicates.
  - comb_W row-sharded (each core computes 128 of 1024 outputs), then
    the x vector is AllGathered (512B).
  - GRU Wih/Whh row-sharded by gate rows; each core computes its 128
    h_new elements, AllGathered (512B).
  - out_W row-sharded (6400 padded vocab rows per core); local
    max/sum-exp stats are AllGathered (8x2 floats) to form the global
    logsumexp; each core writes its log-prob shard.
All fat mat-vecs run on the Vector engine as fused multiply-reduce over
natural-layout weight tiles (PE mat-vecs with K=128 stationary loads are
~0.6us each on fp32 and would bottleneck); the tensor engine only does
cheap K<=18 broadcasts and small transposes.
"""
import sys

sys.path.insert(0, "/opt/trn_rl_repo")

import numpy as np

import concourse.bacc as bacc
import concourse.mybir as mybir
import concourse.tile as tile
from concourse.bass_utils import run_bass_kernel_spmd

H = 1024
L = 18
V = 50257
N_CORES = 8
VP = 6400          # padded vocab rows per core
RP = VP // 128     # 50 rows per partition
GRP = 5            # rows per DMA chunk (per partition)
NCH = RP // GRP    # 10 DMA chunks of out_W per core
F32 = mybir.dt.float32
AX = mybir.AxisListType
AF = mybir.ActivationFunctionType
OP = mybir.AluOpType
NEG_PAD = -1e30

_STATE: dict = {}


def _build_nc():
    nc = bacc.Bacc("TRN2", target_bir_lowering=False, debug=False,
                   num_devices=N_CORES)

    def din(name, shape):
        return nc.dram_tensor(name, shape, F32, kind="ExternalInput")

    def dout(name, shape):
        return nc.dram_tensor(name, shape, F32, kind="ExternalOutput")

    u1b = din("u1b", [L, 2 * H])          # concat(e,h) bcast to 18 parts
    attn_w = din("attn_w", [L, 2 * H])    # attn_W natural
    attn_b_col = din("attn_b_col", [L, 1])
    enc = din("enc", [L, H])
    e_bc = din("e_bc", [128, H])          # embedded row bcast
    h_bc = din("h_bc", [128, H])          # hidden bcast
    h_slice = din("h_slice", [128, 1])    # h[128c:128c+128]
    comb_w = din("comb_w", [128, 2 * H])  # comb_W row shard, natural
    comb_b_col = din("comb_b_col", [128, 1])
    wih_rows = din("wih_rows", [128, 3 * H])  # (p, g*H+k) = Wih[H*g+128c+p, k]
    whh_rows = din("whh_rows", [128, 3 * H])
    bih_col = din("bih_col", [128, 3])
    bhh_col = din("bhh_col", [128, 3])
    out_w = din("out_w", [VP, H])         # padded shard, natural
    out_b_col = din("out_b_col", [128, RP])  # (p, r) = ob[50p + r]
    ident = din("ident", [128, 128])
    ones_row = din("ones_row", [1, 128])

    logp_out = dout("logp", [128, RP])    # (p, r) = logp shard[50p + r]
    hnew_out = dout("hnew", [1, H])       # flat h_new
    attnw_out = dout("attnw", [1, L])

    ag_x_in = nc.dram_tensor("ag_x_in", [128, 1], F32)
    ag_x_out = nc.dram_tensor("ag_x_out", [8 * 128, 1], F32, addr_space="Shared")
    ag_h_in = nc.dram_tensor("ag_h_in", [128, 1], F32)
    ag_h_out = nc.dram_tensor("ag_h_out", [8 * 128, 1], F32, addr_space="Shared")
    ag_s_in = nc.dram_tensor("ag_s_in", [1, 2], F32)
    ag_s_out = nc.dram_tensor("ag_s_out", [N_CORES, 2], F32, addr_space="Shared")

    RG = [list(range(N_CORES))]

    with tile.TileContext(nc) as tc:
        with (
            tc.tile_pool(name="const", bufs=1) as const,
            tc.tile_pool(name="wpool", bufs=4) as wpool,
            tc.tile_pool(name="sb1", bufs=1) as sb1,
            tc.tile_pool(name="scr", bufs=3) as scr,
            tc.tile_pool(name="ptmp", bufs=2, space="PSUM") as ptmp,
            tc.tile_pool(name="pbig", bufs=2, space="PSUM") as pbig,
        ):
            def cload(dram, shape, tag):
                t = const.tile(shape, F32, tag=tag, name=tag)
                nc.sync.dma_start(t[:], dram.ap())
                return t

            # ---- small constants, in critical-path order ----
            u1b_sb = cload(u1b, [L, 2 * H], "u1b_sb")
            aw_sb = cload(attn_w, [L, 2 * H], "aw_sb")
            ab_sb = cload(attn_b_col, [L, 1], "ab_sb")
            enc_sb = cload(enc, [L, H], "enc_sb")
            id_sb = cload(ident, [128, 128], "id_sb")
            on_sb = cload(ones_row, [1, 128], "on_sb")
            e_sb = cload(e_bc, [128, H], "e_sb")
            cw_sb = cload(comb_w, [128, 2 * H], "cw_sb")
            cb_sb = cload(comb_b_col, [128, 1], "cb_sb")
            h_sb = cload(h_bc, [128, H], "h_sb")
            whh_sb = cload(whh_rows, [128, 3 * H], "whh_sb")
            wih_sb = cload(wih_rows, [128, 3 * H], "wih_sb")
            bih_sb = cload(bih_col, [128, 3], "bih_sb")
            bhh_sb = cload(bhh_col, [128, 3], "bhh_sb")
            hs_sb = cload(h_slice, [128, 1], "hs_sb")
            ob_sb = cload(out_b_col, [128, RP], "ob_sb")

            # ---- attention scores (DVE multiply + reduce) ----
            scr_a = sb1.tile([L, 2 * H], F32, tag="scr_a", name="scr_a")
            scores_r = sb1.tile([L, 1], F32, tag="scores_r", name="scores_r")
            scores = sb1.tile([L, 1], F32, tag="scores", name="scores")
            nc.vector.tensor_mul(scr_a[:], aw_sb[:], u1b_sb[:])
            nc.vector.tensor_reduce(scores_r[:], scr_a[:], axis=AX.X,
                                    op=OP.add)
            nc.vector.tensor_add(scores[:], scores_r[:], ab_sb[:])

            # softmax over 18 scores: transpose to one partition
            st_ps = ptmp.tile([1, L], F32, tag="tmp", name="st_ps")
            nc.tensor.matmul(st_ps[:], scores[:], id_sb[:L, :L],
                             is_transpose=True)
            st_sb = sb1.tile([1, L], F32, tag="st_sb", name="st_sb")
            nc.vector.tensor_copy(st_sb[:], st_ps[:])
            negm = sb1.tile([1, 1], F32, tag="negm", name="negm")
            nc.vector.tensor_reduce(negm[:], st_sb[:], axis=AX.X, op=OP.max,
                                    negate=True)
            expv = sb1.tile([1, L], F32, tag="expv", name="expv")
            nc.scalar.activation(expv[:], st_sb[:], AF.Exp, bias=negm[:])
            ssum = sb1.tile([1, 1], F32, tag="ssum", name="ssum")
            nc.vector.reduce_sum(ssum[:], expv[:], axis=AX.X)
            sinv = sb1.tile([1, 1], F32, tag="sinv", name="sinv")
            nc.vector.reciprocal(sinv[:], ssum[:])
            attnw = sb1.tile([1, L], F32, tag="attnw", name="attnw")
            nc.vector.tensor_scalar_mul(attnw[:], expv[:], sinv[:])
            nc.gpsimd.dma_start(attnw_out.ap(), attnw[:])

            # attn_applied broadcast to all partitions, built on PE:
            # rep[l, m] = attnw[l]; aa_bc[p, k] = sum_l rep[l, p] enc[l, k]
            rep_ps = ptmp.tile([L, 128], F32, tag="tmp", name="rep_ps")
            nc.tensor.matmul(rep_ps[:], attnw[:], on_sb[:], start=True,
                             stop=True)
            rep_sb = sb1.tile([L, 128], F32, tag="rep_sb", name="rep_sb")
            nc.vector.tensor_copy(rep_sb[:], rep_ps[:])
            aab_ps = pbig.tile([128, H], F32, tag="big", name="aab_ps")
            for j in range(2):
                nc.tensor.matmul(aab_ps[:, 512 * j:512 * (j + 1)], rep_sb[:],
                                 enc_sb[:, 512 * j:512 * (j + 1)],
                                 start=True, stop=True)
            aab_sb = sb1.tile([128, H], F32, tag="aab_sb", name="aab_sb")
            nc.vector.tensor_copy(aab_sb[:], aab_ps[:])

            # ---- gh gates early (independent of attention/comb) ----
            ghr = sb1.tile([128, 3], F32, tag="ghr", name="ghr")
            gh_sb = sb1.tile([128, 3], F32, tag="gh_sb", name="gh_sb")
            for g in range(3):
                scr_t = scr.tile([128, H], F32, tag="scr", name="scr_t")
                nc.vector.tensor_mul(scr_t[:], whh_sb[:, H * g:H * (g + 1)],
                                     h_sb[:])
                nc.vector.tensor_reduce(ghr[:, g:g + 1], scr_t[:], axis=AX.X,
                                        op=OP.add)
            nc.vector.tensor_add(gh_sb[:], ghr[:], bhh_sb[:])

            # ---- combine + relu ----
            acc_e = sb1.tile([128, 1], F32, tag="acc_e", name="acc_e")
            scr_c = scr.tile([128, H], F32, tag="scr", name="scr_c")
            nc.vector.tensor_mul(scr_c[:], cw_sb[:, 0:H], e_sb[:])
            nc.vector.tensor_reduce(acc_e[:], scr_c[:], axis=AX.X, op=OP.add)
            acc_a = sb1.tile([128, 1], F32, tag="acc_a", name="acc_a")
            scr_d = scr.tile([128, H], F32, tag="scr", name="scr_d")
            nc.vector.tensor_mul(scr_d[:], cw_sb[:, H:2 * H], aab_sb[:])
            nc.vector.tensor_reduce(acc_a[:], scr_d[:], axis=AX.X, op=OP.add)
            x_pre = sb1.tile([128, 1], F32, tag="x_pre", name="x_pre")
            nc.vector.tensor_add(x_pre[:], acc_e[:], acc_a[:])
            x_sb = sb1.tile([128, 1], F32, tag="x_sb", name="x_sb")
            nc.scalar.activation(x_sb[:], x_pre[:], AF.Relu, bias=cb_sb[:])

            # ---- AllGather x (512B), broadcast to all partitions ----
            nc.gpsimd.dma_start(ag_x_in.ap(), x_sb[:])
            nc.gpsimd.collective_compute(
                "AllGather", OP.bypass, replica_groups=RG,
                ins=[ag_x_in.ap().opt()], outs=[ag_x_out.ap().opt()])
            xr_sb = sb1.tile([1, H], F32, tag="xr_sb", name="xr_sb")
            nc.gpsimd.dma_start(
                xr_sb[:], ag_x_out.ap().rearrange("(a b) o -> a (b o)", a=1))
            xbc_ps = pbig.tile([128, H], F32, tag="big", name="xbc_ps")
            for j in range(2):
                nc.tensor.matmul(xbc_ps[:, 512 * j:512 * (j + 1)], on_sb[:],
                                 xr_sb[:, 512 * j:512 * (j + 1)],
                                 start=True, stop=True)
            xbc_sb = sb1.tile([128, H], F32, tag="xbc_sb", name="xbc_sb")
            nc.vector.tensor_copy(xbc_sb[:], xbc_ps[:])

            # ---- gi gates + GRU cell (this core's 128 h_new elements) ----
            gir = sb1.tile([128, 3], F32, tag="gir", name="gir")
            gi_sb = sb1.tile([128, 3], F32, tag="gi_sb", name="gi_sb")
            for g in range(3):
                scr_i = scr.tile([128, H], F32, tag="scr", name="scr_i")
                nc.vector.tensor_mul(scr_i[:], wih_sb[:, H * g:H * (g + 1)],
                                     xbc_sb[:])
                nc.vector.tensor_reduce(gir[:, g:g + 1], scr_i[:], axis=AX.X,
                                        op=OP.add)
            nc.vector.tensor_add(gi_sb[:], gir[:], bih_sb[:])
            rz_in = sb1.tile([128, 2], F32, tag="rz_in", name="rz_in")
            nc.vector.tensor_add(rz_in[:], gi_sb[:, 0:2], gh_sb[:, 0:2])
            rz = sb1.tile([128, 2], F32, tag="rz", name="rz")
            nc.scalar.activation(rz[:], rz_in[:], AF.Sigmoid)
            rh = sb1.tile([128, 1], F32, tag="rh", name="rh")
            nc.vector.tensor_mul(rh[:], rz[:, 0:1], gh_sb[:, 2:3])
            n_in = sb1.tile([128, 1], F32, tag="n_in", name="n_in")
            nc.vector.tensor_add(n_in[:], gi_sb[:, 2:3], rh[:])
            n_t = sb1.tile([128, 1], F32, tag="n_t", name="n_t")
            nc.scalar.activation(n_t[:], n_in[:], AF.Tanh)
            hmn = sb1.tile([128, 1], F32, tag="hmn", name="hmn")
            nc.vector.tensor_sub(hmn[:], hs_sb[:], n_t[:])
            zh = sb1.tile([128, 1], F32, tag="zh", name="zh")
            nc.vector.tensor_mul(zh[:], rz[:, 1:2], hmn[:])
            hn_c = sb1.tile([128, 1], F32, tag="hn_c", name="hn_c")
            nc.vector.tensor_add(hn_c[:], n_t[:], zh[:])

            # ---- AllGather h_new (512B), broadcast to all partitions ----
            nc.gpsimd.dma_start(ag_h_in.ap(), hn_c[:])
            nc.gpsimd.collective_compute(
                "AllGather", OP.bypass, replica_groups=RG,
                ins=[ag_h_in.ap().opt()], outs=[ag_h_out.ap().opt()])
            hr_sb = sb1.tile([1, H], F32, tag="hr_sb", name="hr_sb")
            nc.gpsimd.dma_start(
                hr_sb[:], ag_h_out.ap().rearrange("(a b) o -> a (b o)", a=1))
            nc.gpsimd.dma_start(hnew_out.ap(), hr_sb[:])
            hbc_ps = pbig.tile([128, H], F32, tag="big", name="hbc_ps")
            for j in range(2):
                nc.tensor.matmul(hbc_ps[:, 512 * j:512 * (j + 1)], on_sb[:],
                                 hr_sb[:, 512 * j:512 * (j + 1)],
                                 start=True, stop=True)
            hbc_sb = sb1.tile([128, H], F32, tag="hbc_sb", name="hbc_sb")
            nc.vector.tensor_copy(hbc_sb[:], hbc_ps[:])

            # ---- vocab projection: logits[50p + r] on this core's shard ----
            lraw = sb1.tile([128, RP], F32, tag="lraw", name="lraw")
            wview = out_w.ap().rearrange("(p r) h -> p (r h)", p=128)
            for g in range(NCH):
                wt = wpool.tile([128, GRP * H], F32, tag="w", name="wt")
                nc.sync.dma_start(wt[:], wview[:, GRP * H * g:GRP * H * (g + 1)])
                for r in range(GRP):
                    t = GRP * g + r
                    scr_o = scr.tile([128, H], F32, tag="scr", name="scr_o")
                    nc.vector.tensor_mul(scr_o[:], wt[:, H * r:H * (r + 1)],
                                         hbc_sb[:])
                    nc.scalar.activation(scr_o[:], scr_o[:], AF.Copy,
                                         accum_out=lraw[:, t:t + 1])
            logits_sb = sb1.tile([128, RP], F32, tag="logits_sb",
                                 name="logits_sb")
            nc.vector.tensor_add(logits_sb[:], lraw[:], ob_sb[:])

            # ---- local softmax stats ----
            pmax = sb1.tile([128, 1], F32, tag="pmax", name="pmax")
            nc.vector.tensor_reduce(pmax[:], logits_sb[:], axis=AX.X,
                                    op=OP.max)
            pmt_ps = ptmp.tile([1, 128], F32, tag="tmp", name="pmt_ps")
            nc.tensor.matmul(pmt_ps[:], pmax[:], id_sb[:], is_transpose=True)
            pmt_sb = sb1.tile([1, 128], F32, tag="pmt_sb", name="pmt_sb")
            nc.vector.tensor_copy(pmt_sb[:], pmt_ps[:])
            negmc = sb1.tile([1, 1], F32, tag="negmc", name="negmc")
            nc.vector.tensor_reduce(negmc[:], pmt_sb[:], axis=AX.X, op=OP.max,
                                    negate=True)
            nmb_ps = ptmp.tile([128, 1], F32, tag="tmp", name="nmb_ps")
            nc.tensor.matmul(nmb_ps[:], on_sb[:], negmc[:], start=True,
                             stop=True)
            nmb_sb = sb1.tile([128, 1], F32, tag="nmb_sb", name="nmb_sb")
            nc.vector.tensor_copy(nmb_sb[:], nmb_ps[:])
            expl = sb1.tile([128, RP], F32, tag="expl", name="expl")
            nc.scalar.activation(expl[:], logits_sb[:], AF.Exp,
                                 bias=nmb_sb[:])
            psum_c = sb1.tile([128, 1], F32, tag="psum_c", name="psum_c")
            nc.vector.reduce_sum(psum_c[:], expl[:], axis=AX.X)
            pst_ps = ptmp.tile([1, 128], F32, tag="tmp", name="pst_ps")
            nc.tensor.matmul(pst_ps[:], psum_c[:], id_sb[:], is_transpose=True)
            pst_sb = sb1.tile([1, 128], F32, tag="pst_sb", name="pst_sb")
            nc.vector.tensor_copy(pst_sb[:], pst_ps[:])
            s_c = sb1.tile([1, 1], F32, tag="s_c", name="s_c")
            nc.vector.reduce_sum(s_c[:], pst_sb[:], axis=AX.X)

            # ---- AllGather (m_c, s_c); global lse = LSE_c(m_c + ln s_c) ----
            stats = sb1.tile([1, 2], F32, tag="stats", name="stats")
            nc.vector.tensor_scalar_mul(stats[:, 0:1], negmc[:], -1.0)
            nc.vector.tensor_copy(stats[:, 1:2], s_c[:])
            nc.gpsimd.dma_start(ag_s_in.ap(), stats[:])
            nc.gpsimd.collective_compute(
                "AllGather", OP.bypass, replica_groups=RG,
                ins=[ag_s_in.ap().opt()], outs=[ag_s_out.ap().opt()])
            allst = sb1.tile([N_CORES, 2], F32, tag="allst", name="allst")
            nc.gpsimd.dma_start(allst[:], ag_s_out.ap())
            lns = sb1.tile([N_CORES, 1], F32, tag="lns", name="lns")
            nc.scalar.activation(lns[:], allst[:, 1:2], AF.Ln)
            tvec = sb1.tile([N_CORES, 1], F32, tag="tvec", name="tvec")
            nc.vector.tensor_add(tvec[:], allst[:, 0:1], lns[:])
            tvt_ps = ptmp.tile([1, N_CORES], F32, tag="tmp", name="tvt_ps")
            nc.tensor.matmul(tvt_ps[:], tvec[:], id_sb[:N_CORES, :N_CORES],
                             is_transpose=True)
            tvt_sb = sb1.tile([1, N_CORES], F32, tag="tvt_sb", name="tvt_sb")
            nc.vector.tensor_copy(tvt_sb[:], tvt_ps[:])
            negM = sb1.tile([1, 1], F32, tag="negM", name="negM")
            nc.vector.tensor_reduce(negM[:], tvt_sb[:], axis=AX.X, op=OP.max,
                                    negate=True)
            exv = sb1.tile([1, N_CORES], F32, tag="exv", name="exv")
            nc.scalar.activation(exv[:], tvt_sb[:], AF.Exp, bias=negM[:])
            sex = sb1.tile([1, 1], F32, tag="sex", name="sex")
            nc.vector.reduce_sum(sex[:], exv[:], axis=AX.X)
            lg = sb1.tile([1, 1], F32, tag="lg", name="lg")
            nc.scalar.activation(lg[:], sex[:], AF.Ln)
            neglse = sb1.tile([1, 1], F32, tag="neglse", name="neglse")
            nc.vector.tensor_sub(neglse[:], negM[:], lg[:])
            nlb_ps = ptmp.tile([128, 1], F32, tag="tmp", name="nlb_ps")
            nc.tensor.matmul(nlb_ps[:], on_sb[:], neglse[:], start=True,
                             stop=True)
            nlb_sb = sb1.tile([128, 1], F32, tag="nlb_sb", name="nlb_sb")
            nc.vector.tensor_copy(nlb_sb[:], nlb_ps[:])

            logp_sb = sb1.tile([128, RP], F32, tag="logp_sb", name="logp_sb")
            nc.vector.tensor_scalar_add(logp_sb[:], logits_sb[:], nlb_sb[:])
            nc.sync.dma_start(logp_out.ap(), logp_sb[:])

    nc.compile()
    return nc


def _get_nc():
    if "nc" not in _STATE:
        _STATE["nc"] = _build_nc()
    return _STATE["nc"]


def kernel(input_ids, hidden, encoder_outputs, emb, attn_W, attn_b,
           comb_W, comb_b, gru_Wih, gru_Whh, gru_bih, gru_bhh,
           out_W, out_b):
    nc = _get_nc()
    f = lambda x: np.ascontiguousarray(np.asarray(x, dtype=np.float32))
    input_ids = np.asarray(input_ids)
    idx = int(input_ids.reshape(-1)[0])
    encoder_outputs = f(encoder_outputs)
    attn_W, attn_b = f(attn_W), f(attn_b)
    comb_W, comb_b = f(comb_W), f(comb_b)
    gru_Wih, gru_Whh = f(gru_Wih), f(gru_Whh)
    gru_bih, gru_bhh = f(gru_bih), f(gru_bhh)

    e = f(np.asarray(emb)[idx])            # [H]
    h = f(np.asarray(hidden).reshape(-1))  # [H]
    u1 = np.concatenate([e, h])
    u1b = np.ascontiguousarray(np.broadcast_to(u1, (L, 2 * H)))
    e_bc = np.ascontiguousarray(np.broadcast_to(e, (128, H)))
    h_bc = np.ascontiguousarray(np.broadcast_to(h, (128, H)))
    ident = np.eye(128, dtype=np.float32)
    ones_row = np.ones((1, 128), dtype=np.float32)

    ow = f(out_W)
    ob = f(out_b)
    ow_pad = np.zeros((N_CORES * VP, H), dtype=np.float32)
    ow_pad[:V] = ow
    ob_pad = np.full(N_CORES * VP, NEG_PAD, dtype=np.float32)
    ob_pad[:V] = ob

    def gate_rows(M, c):
        return np.ascontiguousarray(
            np.stack([M[H * g + 128 * c:H * g + 128 * (c + 1)]
                      for g in range(3)], axis=1).reshape(128, -1))

    in_maps = []
    for c in range(N_CORES):
        rows = slice(128 * c, 128 * (c + 1))
        in_maps.append({
            "u1b": u1b, "attn_w": attn_W,
            "attn_b_col": attn_b.reshape(L, 1),
            "enc": encoder_outputs, "e_bc": e_bc, "h_bc": h_bc,
            "h_slice": np.ascontiguousarray(h[rows].reshape(128, 1)),
            "comb_w": np.ascontiguousarray(comb_W[rows]),
            "comb_b_col": comb_b[rows].reshape(128, 1),
            "wih_rows": gate_rows(gru_Wih, c),
            "whh_rows": gate_rows(gru_Whh, c),
            "bih_col": gate_rows(gru_bih.reshape(-1, 1), c),
            "bhh_col": gate_rows(gru_bhh.reshape(-1, 1), c),
            "out_w": np.ascontiguousarray(ow_pad[VP * c:VP * (c + 1)]),
            "out_b_col": np.ascontiguousarray(
                ob_pad[VP * c:VP * (c + 1)].reshape(128, RP)),
            "ident": ident, "ones_row": ones_row,
        })

    res = run_bass_kernel_spmd(nc, in_maps, core_ids=list(range(N_CORES)),
                               **_STATE.get("run_kwargs", {}))
    _STATE["last_results"] = res

    logp = np.concatenate(
        [res.results[c]["logp"].reshape(-1) for c in range(N_CORES)])[:V]
    h_new = res.results[0]["hnew"].reshape(-1)
    attnw = res.results[0]["attnw"].reshape(1, L)
    return (logp.reshape(1, V).astype(np.float32),
            h_new.reshape(1, 1, H).astype(np.float32),
            attnw.astype(np.float32))


# revision 7
# speedup vs baseline: 1.2735x; 1.0122x over previous
"""Trainium2 Bass kernel for a single-step attention-decoder RNN
(embedding lookup -> additive attention -> combine+relu -> GRU cell ->
vocab projection -> log_softmax), tensor-parallel over the vocab dim
across 8 NeuronCores.

Sharding strategy (all host-side prep, one shared SPMD NEFF):
  - embedding lookup is a host-side row slice; the 4KB row replicates.
  - comb_W row-sharded (each core computes 128 of 1024 outputs), then
    the x vector is AllGathered (512B).
  - GRU Wih/Whh row-sharded by gate rows; each core computes its 128
    h_new elements, AllGathered (512B).
  - out_W row-sharded (6400 padded vocab rows per core); local
    max/sum-exp stats are AllGathered (8x2 floats) to form the global
    logsumexp; each core writes its log-prob shard.
All fat mat-vecs run on the Vector engine as fused multiply-reduce over
natural-layout weight tiles (PE mat-vecs with K=128 stationary loads are
~0.6us each on fp32 and would bottleneck); the tensor engine only does
cheap K<=18 broadcasts and small transposes.
"""
import sys

sys.path.insert(0, "/opt/trn_rl_repo")

import numpy as np

import concourse.bacc as bacc
import concourse.mybir as mybir
import concourse.tile as tile
from concourse.bass_utils import run_bass_kernel_spmd

H = 1024
L = 18
V = 50257
N_CORES = 8
VP = 6400          # padded vocab rows per core
RP = VP // 128     # 50 rows per partition
GRP = 5            # rows per DMA chunk (per partition)
NCH = RP // GRP    # 10 DMA chunks of out_W per core
F32 = mybir.dt.float32
AX = mybir.AxisListType
AF = mybir.ActivationFunctionType
OP = mybir.AluOpType
NEG_PAD = -1e30

_STATE: dict = {}


def _build_nc():
    nc = bacc.Bacc("TRN2", target_bir_lowering=False, debug=False,
                   num_devices=N_CORES)

    def din(name, shape):
        return nc.dram_tensor(name, shape, F32, kind="ExternalInput")

    def dout(name, shape):
        return nc.dram_tensor(name, shape, F32, kind="ExternalOutput")

    u1b = din("u1b", [L, 2 * H])          # concat(e,h) bcast to 18 parts
    attn_w = din("attn_w", [L, 2 * H])    # attn_W natural
    attn_b_col = din("attn_b_col", [L, 1])
    enc = din("enc", [L, H])
    e_bc = din("e_bc", [128, H])          # embedded row bcast
    h_bc = din("h_bc", [128, H])          # hidden bcast
    h_slice = din("h_slice", [128, 1])    # h[128c:128c+128]
    comb_w = din("comb_w", [128, 2 * H])  # comb_W row shard, natural
    comb_b_col = din("comb_b_col", [128, 1])
    wih_rows = din("wih_rows", [128, 3 * H])  # (p, g*H+k) = Wih[H*g+128c+p, k]
    whh_rows = din("whh_rows", [128, 3 * H])
    bih_col = din("bih_col", [128, 3])
    bhh_col = din("bhh_col", [128, 3])
    out_w = din("out_w", [VP, H])         # padded shard, natural
    out_b_col = din("out_b_col", [128, RP])  # (p, r) = ob[50p + r]
    ident = din("ident", [128, 128])
    ones_row = din("ones_row", [1, 128])

    logp_out = dout("logp", [128, RP])    # (p, r) = logp shard[50p + r]
    hnew_out = dout("hnew", [1, H])       # flat h_new
    attnw_out = dout("attnw", [1, L])

    ag_x_in = nc.dram_tensor("ag_x_in", [128, 1], F32)
    ag_x_out = nc.dram_tensor("ag_x_out", [8 * 128, 1], F32, addr_space="Shared")
    ag_h_in = nc.dram_tensor("ag_h_in", [128, 1], F32)
    ag_h_out = nc.dram_tensor("ag_h_out", [8 * 128, 1], F32, addr_space="Shared")
    ag_s_in = nc.dram_tensor("ag_s_in", [1, 2], F32)
    ag_s_out = nc.dram_tensor("ag_s_out", [N_CORES, 2], F32, addr_space="Shared")

    RG = [list(range(N_CORES))]

    with tile.TileContext(nc) as tc:
        with (
            tc.tile_pool(name="const", bufs=1) as const,
            tc.tile_pool(name="wpool", bufs=10) as wpool,
            tc.tile_pool(name="sb1", bufs=1) as sb1,
            tc.tile_pool(name="scr", bufs=4) as scr,
            tc.tile_pool(name="ptmp", bufs=2, space="PSUM") as ptmp,
            tc.tile_pool(name="pbig", bufs=2, space="PSUM") as pbig,
        ):
            def cload(dram, shape, tag):
                t = const.tile(shape, F32, tag=tag, name=tag)
                nc.sync.dma_start(t[:], dram.ap())
                return t

            # ---- warm the ACT function tables while DMAs stream ----
            warm_in = sb1.tile([1, 2], F32, tag="warm_in", name="warm_in")
            nc.vector.memset(warm_in[:], 1.0)
            warm_out = sb1.tile([1, 2], F32, tag="warm_out", name="warm_out")
            for fn in (AF.Copy, AF.Exp, AF.Relu, AF.Sigmoid, AF.Tanh, AF.Ln):
                nc.scalar.activation(warm_out[:], warm_in[:], fn)

            # ---- small constants, in critical-path order ----
            u1b_sb = cload(u1b, [L, 2 * H], "u1b_sb")
            aw_sb = cload(attn_w, [L, 2 * H], "aw_sb")
            ab_sb = cload(attn_b_col, [L, 1], "ab_sb")
            enc_sb = cload(enc, [L, H], "enc_sb")
            id_sb = cload(ident, [128, 128], "id_sb")
            on_sb = cload(ones_row, [1, 128], "on_sb")
            e_sb = cload(e_bc, [128, H], "e_sb")
            cw_sb = cload(comb_w, [128, 2 * H], "cw_sb")
            cb_sb = cload(comb_b_col, [128, 1], "cb_sb")
            h_sb = cload(h_bc, [128, H], "h_sb")
            whh_sb = cload(whh_rows, [128, 3 * H], "whh_sb")
            wih_sb = cload(wih_rows, [128, 3 * H], "wih_sb")
            bih_sb = cload(bih_col, [128, 3], "bih_sb")
            bhh_sb = cload(bhh_col, [128, 3], "bhh_sb")
            hs_sb = cload(h_slice, [128, 1], "hs_sb")
            ob_sb = cload(out_b_col, [128, RP], "ob_sb")

            # ---- attention scores (DVE multiply + reduce) ----
            scr_a = sb1.tile([L, 2 * H], F32, tag="scr_a", name="scr_a")
            scores_r = sb1.tile([L, 1], F32, tag="scores_r", name="scores_r")
            scores = sb1.tile([L, 1], F32, tag="scores", name="scores")
            nc.vector.tensor_mul(scr_a[:], aw_sb[:], u1b_sb[:])
            nc.scalar.activation(scr_a[:], scr_a[:], AF.Copy,
                                 accum_out=scores_r[:])
            nc.vector.tensor_add(scores[:], scores_r[:], ab_sb[:])

            # softmax over 18 scores: transpose to one partition
            st_ps = ptmp.tile([1, L], F32, tag="tmp", name="st_ps")
            nc.tensor.matmul(st_ps[:], scores[:], id_sb[:L, :L],
                             is_transpose=True)
            st_sb = sb1.tile([1, L], F32, tag="st_sb", name="st_sb")
            nc.vector.tensor_copy(st_sb[:], st_ps[:])
            negm = sb1.tile([1, 1], F32, tag="negm", name="negm")
            nc.vector.tensor_reduce(negm[:], st_sb[:], axis=AX.X, op=OP.max,
                                    negate=True)
            expv = sb1.tile([1, L], F32, tag="expv", name="expv")
            nc.scalar.activation(expv[:], st_sb[:], AF.Exp, bias=negm[:])
            ssum = sb1.tile([1, 1], F32, tag="ssum", name="ssum")
            nc.vector.reduce_sum(ssum[:], expv[:], axis=AX.X)
            sinv = sb1.tile([1, 1], F32, tag="sinv", name="sinv")
            nc.vector.reciprocal(sinv[:], ssum[:])
            attnw = sb1.tile([1, L], F32, tag="attnw", name="attnw")
            nc.vector.tensor_scalar_mul(attnw[:], expv[:], sinv[:])
            nc.gpsimd.dma_start(attnw_out.ap(), attnw[:])

            # attn_applied broadcast to all partitions, built on PE:
            # rep[l, m] = attnw[l]; aa_bc[p, k] = sum_l rep[l, p] enc[l, k]
            rep_ps = ptmp.tile([L, 128], F32, tag="tmp", name="rep_ps")
            nc.tensor.matmul(rep_ps[:], attnw[:], on_sb[:], start=True,
                             stop=True)
            rep_sb = sb1.tile([L, 128], F32, tag="rep_sb", name="rep_sb")
            nc.vector.tensor_copy(rep_sb[:], rep_ps[:])
            aab_ps = pbig.tile([128, H], F32, tag="big", name="aab_ps")
            for j in range(2):
                nc.tensor.matmul(aab_ps[:, 512 * j:512 * (j + 1)], rep_sb[:],
                                 enc_sb[:, 512 * j:512 * (j + 1)],
                                 start=True, stop=True)
            aab_sb = sb1.tile([128, H], F32, tag="aab_sb", name="aab_sb")
            nc.vector.tensor_copy(aab_sb[:], aab_ps[:])

            # ---- gh gates early (independent of attention/comb) ----
            ghr = sb1.tile([128, 3], F32, tag="ghr", name="ghr")
            gh_sb = sb1.tile([128, 3], F32, tag="gh_sb", name="gh_sb")
            for g in range(3):
                scr_t = scr.tile([128, H], F32, tag="scr", name="scr_t")
                nc.vector.tensor_mul(scr_t[:], whh_sb[:, H * g:H * (g + 1)],
                                     h_sb[:])
                nc.scalar.activation(scr_t[:], scr_t[:], AF.Copy,
                                     accum_out=ghr[:, g:g + 1])
            nc.vector.tensor_add(gh_sb[:], ghr[:], bhh_sb[:])

            # ---- combine + relu ----
            acc_e = sb1.tile([128, 1], F32, tag="acc_e", name="acc_e")
            scr_c = scr.tile([128, H], F32, tag="scr", name="scr_c")
            nc.vector.tensor_mul(scr_c[:], cw_sb[:, 0:H], e_sb[:])
            nc.scalar.activation(scr_c[:], scr_c[:], AF.Copy,
                                 accum_out=acc_e[:])
            acc_a = sb1.tile([128, 1], F32, tag="acc_a", name="acc_a")
            scr_d = scr.tile([128, H], F32, tag="scr", name="scr_d")
            nc.vector.tensor_mul(scr_d[:], cw_sb[:, H:2 * H], aab_sb[:])
            nc.scalar.activation(scr_d[:], scr_d[:], AF.Copy,
                                 accum_out=acc_a[:])
            x_pre = sb1.tile([128, 1], F32, tag="x_pre", name="x_pre")
            nc.vector.tensor_add(x_pre[:], acc_e[:], acc_a[:])
            x_sb = sb1.tile([128, 1], F32, tag="x_sb", name="x_sb")
            nc.scalar.activation(x_sb[:], x_pre[:], AF.Relu, bias=cb_sb[:])

            # ---- AllGather x (512B), broadcast to all partitions ----
            nc.gpsimd.dma_start(ag_x_in.ap(), x_sb[:])
            nc.gpsimd.collective_compute(
                "AllGather", OP.bypass, replica_groups=RG,
                ins=[ag_x_in.ap().opt()], outs=[ag_x_out.ap().opt()])
            xr_sb = sb1.tile([1, H], F32, tag="xr_sb", name="xr_sb")
            nc.gpsimd.dma_start(
                xr_sb[:], ag_x_out.ap().rearrange("(a b) o -> a (b o)", a=1))
            xbc_ps = pbig.tile([128, H], F32, tag="big", name="xbc_ps")
            for j in range(2):
                nc.tensor.matmul(xbc_ps[:, 512 * j:512 * (j + 1)], on_sb[:],
                                 xr_sb[:, 512 * j:512 * (j + 1)],
                                 start=True, stop=True)
            xbc_sb = sb1.tile([128, H], F32, tag="xbc_sb", name="xbc_sb")
            nc.vector.tensor_copy(xbc_sb[:], xbc_ps[:])

            # ---- gi gates + GRU cell (this core's 128 h_new elements) ----
            gir = sb1.tile([128, 3], F32, tag="gir", name="gir")
            gi_sb = sb1.tile([128, 3], F32, tag="gi_sb", name="gi_sb")
            for g in range(3):
                scr_i = scr.tile([128, H], F32, tag="scr", name="scr_i")
                nc.vector.tensor_mul(scr_i[:], wih_sb[:, H * g:H * (g + 1)],
                                     xbc_sb[:])
                nc.scalar.activation(scr_i[:], scr_i[:], AF.Copy,
                                     accum_out=gir[:, g:g + 1])
            nc.vector.tensor_add(gi_sb[:], gir[:], bih_sb[:])
            rz_in = sb1.tile([128, 2], F32, tag="rz_in", name="rz_in")
            nc.vector.tensor_add(rz_in[:], gi_sb[:, 0:2], gh_sb[:, 0:2])
            rz = sb1.tile([128, 2], F32, tag="rz", name="rz")
            nc.scalar.activation(rz[:], rz_in[:], AF.Sigmoid)
            rh = sb1.tile([128, 1], F32, tag="rh", name="rh")
            nc.vector.tensor_mul(rh[:], rz[:, 0:1], gh_sb[:, 2:3])
            n_in = sb1.tile([128, 1], F32, tag="n_in", name="n_in")
            nc.vector.tensor_add(n_in[:], gi_sb[:, 2:3], rh[:])
            n_t = sb1.tile([128, 1], F32, tag="n_t", name="n_t")
            nc.scalar.activation(n_t[:], n_in[:], AF.Tanh)
            hmn = sb1.tile([128, 1], F32, tag="hmn", name="hmn")
            nc.vector.tensor_sub(hmn[:], hs_sb[:], n_t[:])
            zh = sb1.tile([128, 1], F32, tag="zh", name="zh")
            nc.vector.tensor_mul(zh[:], rz[:, 1:2], hmn[:])
            hn_c = sb1.tile([128, 1], F32, tag="hn_c", name="hn_c")
            nc.vector.tensor_add(hn_c[:], n_t[:], zh[:])

            # ---- AllGather h_new (512B), broadcast to all partitions ----
            nc.gpsimd.dma_start(ag_h_in.ap(), hn_c[:])
            nc.gpsimd.collective_compute(
                "AllGather", OP.bypass, replica_groups=RG,
                ins=[ag_h_in.ap().opt()], outs=[ag_h_out.ap().opt()])
            hr_sb = sb1.tile([1, H], F32, tag="hr_sb", name="hr_sb")
            nc.gpsimd.dma_start(
                hr_sb[:], ag_h_out.ap().rearrange("(a b) o -> a (b o)", a=1))
            nc.gpsimd.dma_start(hnew_out.ap(), hr_sb[:])
            hbc_ps = pbig.tile([128, H], F32, tag="big", name="hbc_ps")
            for j in range(2):
                nc.tensor.matmul(hbc_ps[:, 512 * j:512 * (j + 1)], on_sb[:],
                                 hr_sb[:, 512 * j:512 * (j + 1)],
                                 start=True, stop=True)
            hbc_sb = sb1.tile([128, H], F32, tag="hbc_sb", name="hbc_sb")
            nc.vector.tensor_copy(hbc_sb[:], hbc_ps[:])

            # ---- vocab projection: logits[50p + r] on this core's shard ----
            lraw = sb1.tile([128, RP], F32, tag="lraw", name="lraw")
            wview = out_w.ap().rearrange("(p r) h -> p r h", p=128)
            for t in range(RP):
                wt = wpool.tile([128, H], F32, tag="w", name="wt")
                nc.sync.dma_start(wt[:], wview[:, t, :])
                scr_o = scr.tile([128, H], F32, tag="scr", name="scr_o")
                nc.vector.tensor_mul(scr_o[:], wt[:], hbc_sb[:])
                if t % 8 == 0:
                    nc.vector.tensor_reduce(lraw[:, t:t + 1], scr_o[:],
                                            axis=AX.X, op=OP.add)
                else:
                    nc.scalar.activation(scr_o[:], scr_o[:], AF.Copy,
                                         accum_out=lraw[:, t:t + 1])
            logits_sb = sb1.tile([128, RP], F32, tag="logits_sb",
                                 name="logits_sb")
            nc.vector.tensor_add(logits_sb[:], lraw[:], ob_sb[:])

            # ---- local softmax stats ----
            pmax = sb1.tile([128, 1], F32, tag="pmax", name="pmax")
            nc.vector.tensor_reduce(pmax[:], logits_sb[:], axis=AX.X,
                                    op=OP.max)
            pmt_ps = ptmp.tile([1, 128], F32, tag="tmp", name="pmt_ps")
            nc.tensor.matmul(pmt_ps[:], pmax[:], id_sb[:], is_transpose=True)
            pmt_sb = sb1.tile([1, 128], F32, tag="pmt_sb", name="pmt_sb")
            nc.vector.tensor_copy(pmt_sb[:], pmt_ps[:])
            negmc = sb1.tile([1, 1], F32, tag="negmc", name="negmc")
            nc.vector.tensor_reduce(negmc[:], pmt_sb[:], axis=AX.X, op=OP.max,
                                    negate=True)
            nmb_ps = ptmp.tile([128, 1], F32, tag="tmp", name="nmb_ps")
            nc.tensor.matmul(nmb_ps[:], on_sb[:], negmc[:], start=True,
                             stop=True)
            nmb_sb = sb1.tile([128, 1], F32, tag="nmb_sb", name="nmb_sb")
            nc.vector.tensor_copy(nmb_sb[:], nmb_ps[:])
            expl = sb1.tile([128, RP], F32, tag="expl", name="expl")
            nc.scalar.activation(expl[:], logits_sb[:], AF.Exp,
                                 bias=nmb_sb[:])
            psum_c = sb1.tile([128, 1], F32, tag="psum_c", name="psum_c")
            nc.vector.reduce_sum(psum_c[:], expl[:], axis=AX.X)
            pst_ps = ptmp.tile([1, 128], F32, tag="tmp", name="pst_ps")
            nc.tensor.matmul(pst_ps[:], psum_c[:], id_sb[:], is_transpose=True)
            pst_sb = sb1.tile([1, 128], F32, tag="pst_sb", name="pst_sb")
            nc.vector.tensor_copy(pst_sb[:], pst_ps[:])
            s_c = sb1.tile([1, 1], F32, tag="s_c", name="s_c")
            nc.vector.reduce_sum(s_c[:], pst_sb[:], axis=AX.X)

            # ---- AllGather (m_c, s_c); global lse = LSE_c(m_c + ln s_c) ----
            stats = sb1.tile([1, 2], F32, tag="stats", name="stats")
            nc.vector.tensor_scalar_mul(stats[:, 0:1], negmc[:], -1.0)
            nc.vector.tensor_copy(stats[:, 1:2], s_c[:])
            nc.gpsimd.dma_start(ag_s_in.ap(), stats[:])
            nc.gpsimd.collective_compute(
                "AllGather", OP.bypass, replica_groups=RG,
                ins=[ag_s_in.ap().opt()], outs=[ag_s_out.ap().opt()])
            allst = sb1.tile([N_CORES, 2], F32, tag="allst", name="allst")
            nc.gpsimd.dma_start(allst[:], ag_s_out.ap())
            lns = sb1.tile([N_CORES, 1], F32, tag="lns", name="lns")
            nc.scalar.activation(lns[:], allst[:, 1:2], AF.Ln)
            tvec = sb1.tile([N_CORES, 1], F32, tag="tvec", name="tvec")
            nc.vector.tensor_add(tvec[:], allst[:, 0:1], lns[:])
            tvt_ps = ptmp.tile([1, N_CORES], F32, tag="tmp", name="tvt_ps")
            nc.tensor.matmul(tvt_ps[:], tvec[:], id_sb[:N_CORES, :N_CORES],
                             is_transpose=True)
            tvt_sb = sb1.tile([1, N_CORES], F32, tag="tvt_sb", name="tvt_sb")
            nc.vector.tensor_copy(tvt_sb[:], tvt_ps[:])
            negM = sb1.tile([1, 1], F32, tag="negM", name="negM")
            nc.vector.tensor_reduce(negM[:], tvt_sb[:], axis=AX.X, op=OP.max,
                                    negate=True)
            exv = sb1.tile([1, N_CORES], F32, tag="exv", name="exv")
            nc.scalar.activation(exv[:], tvt_sb[:], AF.Exp, bias=negM[:])
            sex = sb1.tile([1, 1], F32, tag="sex", name="sex")
            nc.vector.reduce_sum(sex[:], exv[:], axis=AX.X)
            lg = sb1.tile([1, 1], F32, tag="lg", name="lg")
            nc.scalar.activation(lg[:], sex[:], AF.Ln)
            neglse = sb1.tile([1, 1], F32, tag="neglse", name="neglse")
            nc.vector.tensor_sub(neglse[:], negM[:], lg[:])
            nlb_ps = ptmp.tile([128, 1], F32, tag="tmp", name="nlb_ps")
            nc.tensor.matmul(nlb_ps[:], on_sb[:], neglse[:], start=True,
                             stop=True)
            nlb_sb = sb1.tile([128, 1], F32, tag="nlb_sb", name="nlb_sb")
            nc.vector.tensor_copy(nlb_sb[:], nlb_ps[:])

            logp_sb = sb1.tile([128, RP], F32, tag="logp_sb", name="logp_sb")
            nc.vector.tensor_scalar_add(logp_sb[:], logits_sb[:], nlb_sb[:])
            nc.sync.dma_start(logp_out.ap(), logp_sb[:])

    nc.compile()
    return nc


def _get_nc():
    if "nc" not in _STATE:
        _STATE["nc"] = _build_nc()
    return _STATE["nc"]


def kernel(input_ids, hidden, encoder_outputs, emb, attn_W, attn_b,
           comb_W, comb_b, gru_Wih, gru_Whh, gru_bih, gru_bhh,
           out_W, out_b):
    nc = _get_nc()
    f = lambda x: np.ascontiguousarray(np.asarray(x, dtype=np.float32))
    input_ids = np.asarray(input_ids)
    idx = int(input_ids.reshape(-1)[0])
    encoder_outputs = f(encoder_outputs)
    attn_W, attn_b = f(attn_W), f(attn_b)
    comb_W, comb_b = f(comb_W), f(comb_b)
    gru_Wih, gru_Whh = f(gru_Wih), f(gru_Whh)
    gru_bih, gru_bhh = f(gru_bih), f(gru_bhh)

    e = f(np.asarray(emb)[idx])            # [H]
    h = f(np.asarray(hidden).reshape(-1))  # [H]
    u1 = np.concatenate([e, h])
    u1b = np.ascontiguousarray(np.broadcast_to(u1, (L, 2 * H)))
    e_bc = np.ascontiguousarray(np.broadcast_to(e, (128, H)))
    h_bc = np.ascontiguousarray(np.broadcast_to(h, (128, H)))
    ident = np.eye(128, dtype=np.float32)
    ones_row = np.ones((1, 128), dtype=np.float32)

    ow = f(out_W)
    ob = f(out_b)
    ow_pad = np.zeros((N_CORES * VP, H), dtype=np.float32)
    ow_pad[:V] = ow
    ob_pad = np.full(N_CORES * VP, NEG_PAD, dtype=np.float32)
    ob_pad[:V] = ob

    def gate_rows(M, c):
        return np.ascontiguousarray(
            np.stack([M[H * g + 128 * c:H * g + 128 * (c + 1)]
                      for g in range(3)], axis=1).reshape(128, -1))

    in_maps = []
    for c in range(N_CORES):
        rows = slice(128 * c, 128 * (c + 1))
        in_maps.append({
            "u1b": u1b, "attn_w": attn_W,
            "attn_b_col": attn_b.reshape(L, 1),
            "enc": encoder_outputs, "e_bc": e_bc, "h_bc": h_bc,
            "h_slice": np.ascontiguousarray(h[rows].reshape(128, 1)),
            "comb_w": np.ascontiguousarray(comb_W[rows]),
            "comb_b_col": comb_b[rows].reshape(128, 1),
            "wih_rows": gate_rows(gru_Wih, c),
            "whh_rows": gate_rows(gru_Whh, c),
            "bih_col": gate_rows(gru_bih.reshape(-1, 1), c),
            "bhh_col": gate_rows(gru_bhh.reshape(-1, 1), c),
            "out_w": np.ascontiguousarray(ow_pad[VP * c:VP * (c + 1)]),
            "out_b_col": np.ascontiguousarray(
                ob_pad[VP * c:VP * (c + 1)].reshape(128, RP)),
            "ident": ident, "ones_row": ones_row,
        })

    res = run_bass_kernel_spmd(nc, in_maps, core_ids=list(range(N_CORES)),
                               **_STATE.get("run_kwargs", {}))
    _STATE["last_results"] = res

    logp = np.concatenate(
        [res.results[c]["logp"].reshape(-1) for c in range(N_CORES)])[:V]
    h_new = res.results[0]["hnew"].reshape(-1)
    attnw = res.results[0]["attnw"].reshape(1, L)
    return (logp.reshape(1, V).astype(np.float32),
            h_new.reshape(1, 1, H).astype(np.float32),
            attnw.astype(np.float32))
